# revision 2
# baseline (speedup 1.0000x reference)
"""Trainium2 8-core attention kernel v2 for nn_Attention_8409545965959.

Reference computation (B=4, N=2048, C=1024, H=16 heads, Dh=64):
    qkv = x @ Wqkv; q,k,v per head
    att = softmax(where(mask>0, -1e7, q @ k^T / sqrt(Dh)))
    out = (att @ v) @ Wproj + bproj

Sharding: tensor-parallel on heads (2 heads/core, column-parallel Wqkv,
row-parallel Wproj). NO collectives: each core computes the row-parallel
proj PARTIAL over its 128 channels for the FULL [8192, 1024] output; the
host sums the 8 partials and adds the bias (the unshard combine).

Masked keys contribute exactly zero to the softmax, so K/V are compacted
host-side to the unmasked tokens of each batch, padded to a multiple of
128 (padded positions re-masked on device via the exp bias).

On-device dataflow (per core, heads h0=2c, h1=2c+1):
  - qT/kT [128ch, n] from Wq/Wk-stationary matmuls vs host-transposed
    x^T; v in [token, ch] layout.
  - S^T[k,q] per head via row-group-packed matmul pairs (K=Dh=64),
    both heads' scores in one PSUM tile [128, 1024].
  - softmax: exp via ScalarE activation (scale=1/sqrt(Dh), per-partition
    bias = -30000 on masked/padded k rows), E^T in bf16.
  - O^T += v_h^T @ E^T col-group-packed into one PSUM bank; softmax
    denominators D via N=1 matmuls (stationary = E^T q-chunk, moving =
    ones column) -> [128q, 1] per (head, q-chunk), nearly free.
  - normalization: reciprocal on D [128,8], PE-transpose to [8,128],
    broadcast K=1 matmuls -> [128ch, 512q], multiply on VectorE.
  - proj: Wproj-slice-stationary (K = our 128 channels, single chunk)
    per 128-out-channel block; PSUM->SBUF bf16 copies on the Pool
    engine; DMA partial^T [1024, 8192] bf16 to DRAM.

Emission interleaves the next batch's QKV matmuls and the previous
q-block's norm/proj into the attention inner loop so the PE never idles
(the cost model's p-state throttle lifts only after 3us of gap-free PE
activity). O/D matmuls are emitted lagged one kc-step behind scores so
they never head-of-line block on the exp.

kernel(**inputs) accepts the full unsharded inputs and returns the full
[4, 2048, 1024] float32 output.
"""

import sys
import types

import numpy as np
import ml_dtypes

try:
    import antenv.axon_hooks  # noqa: F401
except ImportError:
    try:
        import antenv

        _ah = types.ModuleType("antenv.axon_hooks")
        _ah._hook = None
        _ah.set_axon_ntff_profile_hook = lambda h: setattr(_ah, "_hook", h)
        _ah.get_axon_ntff_profile_hook = lambda: _ah._hook
        sys.modules["antenv.axon_hooks"] = _ah
        antenv.axon_hooks = _ah
    except ImportError:
        pass

import concourse.bass as bass  # noqa: F401
import concourse.mybir as mybir
import concourse.tile as tile
from concourse import bacc
from concourse.bass_utils import run_bass_kernel_spmd

B = 4
N = 2048
C = 1024
H = 16
NCORES = 8
DH = C // H            # 64
HPC = H // NCORES      # 2 heads per core -> 128 channels/core
CPC = HPC * DH         # 128
ROWS = B * N           # 8192
QB = 512               # q block (one PSUM bank of f32)
KCH = 128              # k chunk (partitions)
NQB = N // QB          # 4
CC = C // 128          # 8 contraction chunks
SCALE = DH ** -0.5     # 0.125
MASK_BIAS = -30000.0

DT = mybir.dt.float32
BF = mybir.dt.bfloat16
NPBF = ml_dtypes.bfloat16

_CACHE: dict = {}
LAST_RESULTS = None


def _build(nkcs):
    """nkcs = per-batch number of 128-row k-chunks after compaction."""
    nkt = sum(nkcs)                      # total k chunks across batches
    nko = [sum(nkcs[:b]) for b in range(B)]   # chunk offset per batch
    nk_max = max(nkcs) * KCH
    nc = bacc.Bacc("TRN2", target_bir_lowering=False, debug=False, num_devices=NCORES)

    xT = nc.dram_tensor("xT", [C, ROWS], BF, kind="ExternalInput")
    xTk = nc.dram_tensor("xTk", [C, nkt * KCH], BF, kind="ExternalInput")
    wq = nc.dram_tensor("wq", [C, CPC], BF, kind="ExternalInput")
    wk = nc.dram_tensor("wk", [C, CPC], BF, kind="ExternalInput")
    wv = nc.dram_tensor("wv", [C, CPC], BF, kind="ExternalInput")
    wps = nc.dram_tensor("wps", [CPC, C], BF, kind="ExternalInput")
    mb = nc.dram_tensor("mb", [128, nkt], DT, kind="ExternalInput")
    ident = nc.dram_tensor("ident", [128, 128], DT, kind="ExternalInput")
    out_ext = nc.dram_tensor("out", [C, ROWS], BF, kind="ExternalOutput")

    # k blocks for the K^T qkv matmuls (moving dim <= 512), per batch
    kblocks_b = []
    for b in range(B):
        kb_ = []
        pos = 0
        while pos < nkcs[b] * KCH:
            w = min(QB, nkcs[b] * KCH - pos)
            kb_.append((pos, w))
            pos += w
        kblocks_b.append(kb_)

    with tile.TileContext(nc) as tc:
        with (
            tc.tile_pool(name="consts", bufs=1) as consts,
            tc.tile_pool(name="xpool", bufs=2) as xpool,
            tc.tile_pool(name="kpool", bufs=2) as kpool,
            tc.tile_pool(name="qkpool", bufs=2) as qkpool,
            tc.tile_pool(name="vpool", bufs=2) as vpool,
            tc.tile_pool(name="epool", bufs=4) as epool,
            tc.tile_pool(name="npool", bufs=4) as npool,
            tc.tile_pool(name="opool", bufs=3) as opool,
            tc.tile_pool(name="outp", bufs=4) as outp,
            tc.tile_pool(name="s_ps", bufs=2, space="PSUM") as s_ps,
            tc.tile_pool(name="o_ps", bufs=1, space="PSUM") as o_ps,
            tc.tile_pool(name="d_ps", bufs=1, space="PSUM") as d_ps,
            tc.tile_pool(name="aux_ps", bufs=2, space="PSUM") as aux_ps,
        ):
            # ---- persistent constants / weights
            wq_sb = consts.tile([128, CC, CPC], BF)
            wk_sb = consts.tile([128, CC, CPC], BF)
            wv_sb = consts.tile([128, CC, CPC], BF)
            wp_sb = consts.tile([128, CC, 128], BF)
            mb_sb = consts.tile([128, nkt], DT)
            id_sb = consts.tile([128, 128], DT)
            ones_sb = consts.tile([128, 1], BF)
            ones64 = consts.tile([1, 64], BF)
            nc.sync.dma_start(wq_sb[:], wq.rearrange("(cc p) m -> p cc m", p=128))
            nc.sync.dma_start(wk_sb[:], wk.rearrange("(cc p) m -> p cc m", p=128))
            nc.sync.dma_start(wv_sb[:], wv.rearrange("(cc p) m -> p cc m", p=128))
            nc.sync.dma_start(wp_sb[:], wps.rearrange("p (oc m) -> p oc m", m=128))
            nc.sync.dma_start(mb_sb[:], mb[:])
            nc.sync.dma_start(id_sb[:], ident[:])
            nc.vector.memset(ones_sb[:], 1.0)
            nc.vector.memset(ones64[:], 1.0)


            xb_tiles = {}
            kb_tiles = {}
            qkv_state = {}

            def emit_xb_load(b, first=False):
                xb = xpool.tile([128, CC, N], BF, name=f"xb{b}", tag="xb")
                xs = xT[:, b * N:(b + 1) * N].rearrange("(cc p) n -> p cc n", p=128)
                nk = nkcs[b] * KCH
                kb = kpool.tile([128, CC, nk], BF, name=f"kb{b}", tag="kb")
                ks = xTk[:, nko[b] * KCH:nko[b] * KCH + nk].rearrange(
                    "(cc p) n -> p cc n", p=128)
                if first:
                    # column-block-first order: q_unit(0)/k_unit(0) can start
                    # after ~1MB instead of after the full 6.25MB batch load
                    for cc in range(CC):
                        nc.sync.dma_start(kb[:, cc, 0:QB], ks[:, cc, 0:QB])
                    for cc in range(CC):
                        nc.sync.dma_start(xb[:, cc, 0:QB], xs[:, cc, 0:QB])
                    for cc in range(CC):
                        nc.sync.dma_start(kb[:, cc, QB:nk], ks[:, cc, QB:nk])
                    for rb in range(1, NQB):
                        for cc in range(CC):
                            nc.sync.dma_start(
                                xb[:, cc, rb * QB:(rb + 1) * QB],
                                xs[:, cc, rb * QB:(rb + 1) * QB],
                            )
                else:
                    # coarse loads: one dma_start each keeps the Sync engine
                    # (565ns per issue) nearly idle
                    nc.sync.dma_start(xb[:], xs)
                    nc.sync.dma_start(kb[:], ks)
                xb_tiles[b] = xb
                kb_tiles[b] = kb

            def qkv_units(b):
                """Independent emission units for batch b's QKV (filler work)."""
                xb = xb_tiles[b]
                kb = kb_tiles[b]
                nk = nkcs[b] * KCH
                qT = qkpool.tile([128, N], BF, name=f"qT{b}", tag="qT")
                kT = qkpool.tile([128, nk_max], BF, name=f"kT{b}", tag="kT")
                vt = vpool.tile([128, max(nkcs), CPC], BF, name=f"vt{b}", tag="vt")
                qkv_state[b] = (qT, kT, vt)
                units = []

                def k_unit(pos, w):
                    def emit():
                        ps = aux_ps.tile([128, QB], DT, name=f"psk{b}_{pos}", tag="aux")
                        for cc in range(CC):
                            nc.tensor.matmul(
                                ps[:, 0:w],
                                wk_sb[:, cc, :],
                                kb[:, cc, pos:pos + w],
                                start=cc == 0,
                                stop=cc == CC - 1,
                            )
                        nc.vector.tensor_copy(kT[:, pos:pos + w], ps[:, 0:w])

                    return emit

                def q_unit(rb):
                    def emit():
                        ps = aux_ps.tile([128, QB], DT, name=f"psq{b}_{rb}", tag="aux")
                        for cc in range(CC):
                            nc.tensor.matmul(
                                ps[:],
                                wq_sb[:, cc, :],
                                xb[:, cc, rb * QB:(rb + 1) * QB],
                                start=cc == 0,
                                stop=cc == CC - 1,
                            )
                        nc.vector.tensor_copy(qT[:, rb * QB:(rb + 1) * QB], ps[:])

                    return emit

                def v_unit(rc):
                    def emit():
                        ps = aux_ps.tile([128, QB], DT, name=f"psv{b}_{rc}", tag="aux")
                        for cc in range(CC):
                            nc.tensor.matmul(
                                ps[:, 0:CPC],
                                kb[:, cc, rc * KCH:(rc + 1) * KCH],
                                wv_sb[:, cc, :],
                                start=cc == 0,
                                stop=cc == CC - 1,
                            )
                        nc.vector.tensor_copy(vt[:, rc, :], ps[:, 0:CPC])

                    return emit

                # k first (scores need all of kT), then q blocks in order,
                # then v chunks (needed from the first O matmul onwards)
                for pos, w in kblocks_b[b]:
                    units.append(k_unit(pos, w))
                units.append(q_unit(0))
                for rc in range(nkcs[b]):
                    units.append(v_unit(rc))
                for rb in range(1, NQB):
                    units.append(q_unit(rb))
                return units

            def attention_steps(b, carried, extra=None):
                """Returns the woven list of emission closures for batch b.

                carried: trailing closures from the previous batch's last
                q-block (norm_b + proj units), woven into qb 0. extra:
                units appended to qb 0's trailers (run during qb 1).
                """
                qT, kT, vt = qkv_state[b]
                nkc = nkcs[b]
                o_tiles = {}
                d_tiles = {}
                e_tiles = {}
                nstate = {}
                of_tiles = {}

                def kc_scores(qb, kc):
                    def emit():
                        if kc == 0:
                            o_tiles[qb] = o_ps.tile(
                                [128, QB], DT, name=f"o{b}_{qb}", tag="o"
                            )
                            d_tiles[qb] = d_ps.tile(
                                [128, 256], DT, name=f"d{b}_{qb}", tag="d"
                            )
                        s2 = s_ps.tile(
                            [128, 2 * QB], DT, name=f"s{b}_{qb}_{kc}", tag="s"
                        )
                        nc.tensor.matmul(
                            s2[:, 0:QB],
                            kT[0:DH, kc * KCH:(kc + 1) * KCH],
                            qT[0:DH, qb * QB:(qb + 1) * QB],
                            start=True,
                            stop=True,
                            tile_position=(0, 0),
                        )
                        nc.tensor.matmul(
                            s2[:, QB:2 * QB],
                            kT[DH:2 * DH, kc * KCH:(kc + 1) * KCH],
                            qT[DH:2 * DH, qb * QB:(qb + 1) * QB],
                            start=True,
                            stop=True,
                            tile_position=(64, 0),
                        )
                        e2 = epool.tile(
                            [128, 2 * QB], BF, name=f"e{b}_{qb}_{kc}", tag="e"
                        )
                        mcol = nko[b] + kc
                        nc.scalar.activation(
                            e2[:],
                            s2[:],
                            mybir.ActivationFunctionType.Exp,
                            bias=mb_sb[:, mcol:mcol + 1],
                            scale=SCALE,
                        )
                        e_tiles[(qb, kc)] = e2

                    return emit

                def kc_ov(qb, kc):
                    def emit():
                        e2 = e_tiles.pop((qb, kc))
                        o_acc = o_tiles[qb]
                        d = d_tiles[qb]
                        st = kc == 0
                        sp = kc == nkc - 1
                        nc.tensor.matmul(
                            o_acc[0:DH, :],
                            vt[:, kc, 0:DH],
                            e2[:, 0:QB],
                            start=st,
                            stop=sp,
                            tile_position=(0, 0),
                        )
                        nc.tensor.matmul(
                            o_acc[DH:2 * DH, :],
                            vt[:, kc, DH:2 * DH],
                            e2[:, QB:2 * QB],
                            start=st,
                            stop=sp,
                            tile_position=(0, 64),
                        )
                        # PSUM start=True zeroes the whole 2KB bank row, so
                        # only the FIRST D group may set it; the rest inherit
                        # the bank-wide pending-zero (skip the group checker)
                        for h in range(2):
                            for qc in range(4):
                                col = h * 4 + qc
                                nc.tensor.matmul(
                                    d[:, col:col + 1],
                                    e2[:, h * QB + qc * 128:h * QB + (qc + 1) * 128],
                                    ones_sb[:],
                                    start=st and col == 0,
                                    stop=sp,
                                    skip_group_check=True,
                                )

                    return emit

                def norm_a(qb):
                    def emit():
                        # free the O PSUM bank immediately; reciprocal of D
                        o_acc = o_tiles.pop(qb)
                        d = d_tiles[qb]
                        rds = npool.tile([128, 8], DT, name=f"rds{b}_{qb}", tag="rds")
                        nc.vector.reciprocal_approx_fast(rds[:], d[:, 0:8])
                        osb = opool.tile([128, QB], DT, name=f"osb{b}_{qb}", tag="osb")
                        nc.vector.tensor_copy(osb[:], o_acc[:])
                        nstate[qb] = (osb, rds)

                    return emit

                def norm_t(qb):
                    def emit():
                        # 8 column transposes [128,1]->[1,128] in f32, all
                        # landing on partition 0 (matmul moving operands must
                        # start at an aligned partition);
                        # 4 regions per bank, assembled to one bf16 row
                        osb, rds = nstate[qb]
                        d_tiles.pop(qb)
                        rdsb = npool.tile(
                            [1, 8 * 128], BF, name=f"rdt{b}_{qb}", tag="rdt"
                        )
                        for half in range(2):
                            tt = aux_ps.tile(
                                [128, QB], DT, name=f"tt{b}_{qb}_{half}", tag="aux"
                            )
                            for i in range(4):
                                r = half * 4 + i
                                nc.tensor.transpose(
                                    tt[0:1, i * 128:(i + 1) * 128],
                                    rds[:, r:r + 1],
                                    id_sb[:],
                                )
                            nc.vector.tensor_copy(
                                rdsb[0:1, half * QB:(half + 1) * QB], tt[0:1, :]
                            )
                        nstate[qb] = (osb, rdsb)

                    return emit

                def norm_b(qb):
                    def emit():
                        osb, rdsb = nstate.pop(qb)
                        brd = aux_ps.tile([128, QB], DT, name=f"brd{b}_{qb}", tag="aux")
                        for qc in range(4):
                            nc.tensor.matmul(
                                brd[0:64, qc * 128:(qc + 1) * 128],
                                ones64[:],
                                rdsb[0:1, qc * 128:(qc + 1) * 128],
                                start=True,
                                stop=True,
                                tile_position=(0, 0),
                            )
                            nc.tensor.matmul(
                                brd[64:128, qc * 128:(qc + 1) * 128],
                                ones64[:],
                                rdsb[0:1, (4 + qc) * 128:(5 + qc) * 128],
                                start=True,
                                stop=True,
                                tile_position=(0, 64),
                            )
                        of = opool.tile([128, QB], BF, name=f"of{b}_{qb}", tag="of")
                        nc.vector.tensor_mul(of[:], osb[:], brd[:])
                        of_tiles[qb] = of
                        # per-qb output staging: 8 oc blocks, one DMA at the end
                        qkv_state[(b, qb, "ob")] = outp.tile(
                            [128, CC, QB], BF, name=f"ob{b}_{qb}", tag="ob"
                        )

                    return emit

                def proj_unit(qb, oc, last):
                    def emit():
                        of = of_tiles[qb] if not last else of_tiles.pop(qb)
                        ob = qkv_state[(b, qb, "ob")]
                        pps = aux_ps.tile(
                            [128, QB], DT, name=f"pp{b}_{qb}_{oc}", tag="aux"
                        )
                        nc.tensor.matmul(
                            pps[:], wp_sb[:, oc, :], of[:], start=True, stop=True
                        )
                        # split PSUM->SBUF copies between Vector and Scalar
                        if oc < 6:
                            nc.vector.tensor_copy(ob[:, oc, :], pps[:])
                        else:
                            nc.scalar.activation(
                                ob[:, oc, :],
                                pps[:],
                                mybir.ActivationFunctionType.Copy,
                            )
                        if last:
                            ob2 = qkv_state.pop((b, qb, "ob"))
                            nc.sync.dma_start(
                                out_ext[:, b * N + qb * QB:b * N + (qb + 1) * QB]
                                .rearrange("(oc p) q -> p oc q", p=128),
                                ob2[:],
                            )

                    return emit

                def qb_trailers(qb):
                    t = [norm_t(qb), norm_b(qb)]
                    for oc in range(CC):
                        t.append(proj_unit(qb, oc, oc == CC - 1))
                    if qb == 0 and extra:
                        t.extend(extra)
                    return t

                # weave: per qb emit scores(kc) with O/D lagged one step;
                # trailing work (norm_b + 8 proj units) of the PREVIOUS
                # q-block lands in slots kc=1..nkc-1 of this q-block.
                woven = []
                trailing = list(carried) if carried else []
                for qb in range(NQB):
                    for kc in range(nkc):
                        woven.append(kc_scores(qb, kc))
                        if kc >= 1:
                            woven.append(kc_ov(qb, kc - 1))
                            if trailing:
                                woven.append(trailing.pop(0))
                    # drain leftover trailing units BEFORE the O/D flush so
                    # the flush never head-of-line blocks on the last exp
                    while trailing:
                        woven.append(trailing.pop(0))
                    woven.append(kc_ov(qb, nkc - 1))
                    woven.append(norm_a(qb))
                    trailing = qb_trailers(qb)
                return woven, trailing

            def run_interleaved(steps, fillers):
                nf = len(fillers)
                ns = len(steps)
                fi = 0
                for i, s in enumerate(steps):
                    s()
                    while fi < nf and (i + 1) * nf >= (fi + 1) * ns:
                        fillers[fi]()
                        fi += 1
                while fi < nf:
                    fillers[fi]()
                    fi += 1

            # ---- schedule
            emit_xb_load(0, first=True)
            for u in qkv_units(0):
                u()
            carried = None
            for b in range(B):
                fillers = []
                if b < B - 1:
                    emit_xb_load(b + 1)
                    fillers.extend(qkv_units(b + 1))
                steps, carried = attention_steps(b, carried)
                run_interleaved(steps, fillers)
            # tail: last q-block's norm_b + proj
            for u in carried:
                u()

    nc.compile()
    return nc


def _prep_inputs(x, Wqkv, Wproj, bproj, mask, nkcs):
    x = np.asarray(x, dtype=np.float32)
    Wqkv = np.asarray(Wqkv, dtype=np.float32)
    Wproj = np.asarray(Wproj, dtype=np.float32)
    mask = np.asarray(mask)
    nkt = sum(nkcs)
    nko = [sum(nkcs[:b]) for b in range(B)]

    x2 = x.reshape(ROWS, C)
    xT = np.ascontiguousarray(x2.T).astype(NPBF)
    # compacted K/V tokens: unmasked columns per batch, zero-padded per batch
    xTk = np.zeros((C, nkt * KCH), dtype=NPBF)
    # mb columns: one per (batch, chunk); 128 rows = positions in the chunk
    mb_arr = np.full((128, nkt), np.float32(MASK_BIAS), dtype=np.float32)
    for b in range(B):
        idx = np.nonzero(mask[b] == 0)[0]
        cnt = len(idx)
        xTk[:, nko[b] * KCH: nko[b] * KCH + cnt] = xT[:, b * N + idx]
        mbias = np.full(nkcs[b] * KCH, np.float32(MASK_BIAS), dtype=np.float32)
        mbias[:cnt] = 0.0
        mb_arr[:, nko[b]:nko[b] + nkcs[b]] = mbias.reshape(nkcs[b], 128).T
    ident = np.eye(128, dtype=np.float32)

    in_maps = []
    for c in range(NCORES):
        cols = slice(c * CPC, (c + 1) * CPC)
        in_maps.append(
            dict(
                xT=xT,
                xTk=xTk,
                wq=np.ascontiguousarray(Wqkv[:, cols]).astype(NPBF),
                wk=np.ascontiguousarray(Wqkv[:, C:][:, cols]).astype(NPBF),
                wv=np.ascontiguousarray(Wqkv[:, 2 * C:][:, cols]).astype(NPBF),
                wps=np.ascontiguousarray(Wproj[cols, :]).astype(NPBF),
                mb=mb_arr,
                ident=ident,
            )
        )
    return in_maps


def kernel(x, Wqkv, Wproj, bproj, mask):
    global LAST_RESULTS
    mask = np.asarray(mask)
    bproj = np.asarray(bproj, dtype=np.float32)
    cnts = (mask == 0).sum(axis=1)
    nkcs = tuple(max(1, -(-int(c) // KCH)) for c in cnts)
    if nkcs not in _CACHE:
        _CACHE[nkcs] = _build(nkcs)
    nc = _CACHE[nkcs]
    in_maps = _prep_inputs(x, Wqkv, Wproj, bproj, mask, nkcs)
    res = run_bass_kernel_spmd(nc, in_maps, list(range(NCORES)))
    LAST_RESULTS = res
    # host-side unshard for row-parallel Wproj: sum the 8 partials + bias
    acc = np.zeros((C, ROWS), dtype=np.float32)
    for c in range(NCORES):
        acc += res.results[c]["out"].astype(np.float32)
    out = acc.T + bproj[None, :]
    return np.ascontiguousarray(out).reshape(B, N, C)


# revision 3
# speedup vs baseline: 1.1333x; 1.1333x over previous
"""Trainium2 8-core attention kernel v2 for nn_Attention_8409545965959.

Reference computation (B=4, N=2048, C=1024, H=16 heads, Dh=64):
    qkv = x @ Wqkv; q,k,v per head
    att = softmax(where(mask>0, -1e7, q @ k^T / sqrt(Dh)))
    out = (att @ v) @ Wproj + bproj

Sharding: tensor-parallel on heads (2 heads/core, column-parallel Wqkv,
row-parallel Wproj). NO collectives: each core computes the row-parallel
proj PARTIAL over its 128 channels for the FULL [8192, 1024] output; the
host sums the 8 partials and adds the bias (the unshard combine).

Masked keys contribute exactly zero to the softmax, so K/V are compacted
host-side to the unmasked tokens of each batch, padded per batch to a
multiple of 128 (per-batch chunk counts; padded positions re-masked on
device via the exp bias).

On-device dataflow (per core, heads h0=2c, h1=2c+1):
  - qT/kT [128ch, n] from Wq/Wk-stationary matmuls vs host-transposed
    x^T; v in [token, ch] layout.
  - S^T[k,q] per head via row-group-packed matmul pairs (K=Dh=64),
    both heads' scores in one PSUM tile [128, 1024].
  - softmax: exp via ScalarE activation (scale=1/sqrt(Dh), per-partition
    bias = -30000 on masked/padded k rows), E^T in bf16.
  - O^T += v_h^T @ E^T col-group-packed into one PSUM bank; softmax
    denominators D via N=1 matmuls (stationary = E^T q-chunk, moving =
    ones column) -> [128q, 1] per (head, q-chunk), nearly free in the
    cost model (matmul time scales with output free-size only). Only
    the first D group uses start=True: a PSUM start zeroes the whole
    2KB bank row, which would wipe sibling accumulators.
  - normalization: reciprocal on D [128,8] (VectorE), 8 PE column
    transposes [128,1]->[1,128] landing on partition 0 (matmul moving
    operands must start at an aligned partition), two copy-casts to a
    bf16 [1,1024] row, broadcast K=1 matmuls -> [128ch, 512q] in PSUM,
    multiply on VectorE.
  - proj: Wproj-slice-stationary (K = our 128 channels, single chunk)
    per 128-out-channel block; PSUM->SBUF bf16 copies split 6:2 between
    VectorE and ScalarE; one DMA per q-block writes the partial^T
    [1024, 8192] bf16 to DRAM.

Emission interleaves the next batch's QKV matmuls and the previous
q-block's norm/proj into the attention inner loop so the PE never idles
(the cost model's p-state throttle lifts only after 3us of gap-free PE
activity). O/D matmuls are emitted lagged one kc-step behind scores so
they never head-of-line block on the exp.

kernel(**inputs) accepts the full unsharded inputs and returns the full
[4, 2048, 1024] float32 output.
"""

import sys
import types

import numpy as np
import ml_dtypes

try:
    import antenv.axon_hooks  # noqa: F401
except ImportError:
    try:
        import antenv

        _ah = types.ModuleType("antenv.axon_hooks")
        _ah._hook = None
        _ah.set_axon_ntff_profile_hook = lambda h: setattr(_ah, "_hook", h)
        _ah.get_axon_ntff_profile_hook = lambda: _ah._hook
        sys.modules["antenv.axon_hooks"] = _ah
        antenv.axon_hooks = _ah
    except ImportError:
        pass

import concourse.bass as bass  # noqa: F401
import concourse.mybir as mybir
import concourse.tile as tile
from concourse import bacc
from concourse.bass_utils import run_bass_kernel_spmd

B = 4
N = 2048
C = 1024
H = 16
NCORES = 8
DH = C // H            # 64
HPC = H // NCORES      # 2 heads per core -> 128 channels/core
CPC = HPC * DH         # 128
ROWS = B * N           # 8192
QB = 512               # q block (one PSUM bank of f32)
KCH = 128              # k chunk (partitions)
NQB = N // QB          # 4
CC = C // 128          # 8 contraction chunks
SCALE = DH ** -0.5     # 0.125
MASK_BIAS = -30000.0

DT = mybir.dt.float32
BF = mybir.dt.bfloat16
NPBF = ml_dtypes.bfloat16

_CACHE: dict = {}
LAST_RESULTS = None


def _build(nkcs):
    """nkcs = per-batch number of 128-row k-chunks after compaction."""
    nkt = sum(nkcs)                      # total k chunks across batches
    nko = [sum(nkcs[:b]) for b in range(B)]   # chunk offset per batch
    nk_max = max(nkcs) * KCH
    nc = bacc.Bacc("TRN2", target_bir_lowering=False, debug=False, num_devices=NCORES)

    xT = nc.dram_tensor("xT", [C, ROWS], BF, kind="ExternalInput")
    xTk = nc.dram_tensor("xTk", [C, nkt * KCH], BF, kind="ExternalInput")
    wq = nc.dram_tensor("wq", [C, CPC], BF, kind="ExternalInput")
    wk = nc.dram_tensor("wk", [C, CPC], BF, kind="ExternalInput")
    wv = nc.dram_tensor("wv", [C, CPC], BF, kind="ExternalInput")
    wps = nc.dram_tensor("wps", [CPC, C], BF, kind="ExternalInput")
    mb = nc.dram_tensor("mb", [128, nkt], DT, kind="ExternalInput")
    ident = nc.dram_tensor("ident", [128, 128], DT, kind="ExternalInput")
    out_ext = nc.dram_tensor("out", [C, ROWS], BF, kind="ExternalOutput")

    # k blocks for the K^T qkv matmuls (moving dim <= 512), per batch
    kblocks_b = []
    for b in range(B):
        kb_ = []
        pos = 0
        while pos < nkcs[b] * KCH:
            w = min(QB, nkcs[b] * KCH - pos)
            kb_.append((pos, w))
            pos += w
        kblocks_b.append(kb_)

    with tile.TileContext(nc) as tc:
        with (
            tc.tile_pool(name="consts", bufs=1) as consts,
            tc.tile_pool(name="xpool", bufs=2) as xpool,
            tc.tile_pool(name="kpool", bufs=2) as kpool,
            tc.tile_pool(name="qkpool", bufs=2) as qkpool,
            tc.tile_pool(name="vpool", bufs=2) as vpool,
            tc.tile_pool(name="epool", bufs=4) as epool,
            tc.tile_pool(name="npool", bufs=4) as npool,
            tc.tile_pool(name="opool", bufs=3) as opool,
            tc.tile_pool(name="outp", bufs=4) as outp,
            tc.tile_pool(name="s_ps", bufs=2, space="PSUM") as s_ps,
            tc.tile_pool(name="o_ps", bufs=1, space="PSUM") as o_ps,
            tc.tile_pool(name="d_ps", bufs=1, space="PSUM") as d_ps,
            tc.tile_pool(name="aux_ps", bufs=2, space="PSUM") as aux_ps,
        ):
            # ---- persistent constants / weights
            wq_sb = consts.tile([128, CC, CPC], BF)
            wk_sb = consts.tile([128, CC, CPC], BF)
            wv_sb = consts.tile([128, CC, CPC], BF)
            wp_sb = consts.tile([128, CC, 128], BF)
            mb_sb = consts.tile([128, nkt], DT)
            id_sb = consts.tile([128, 128], DT)
            ones_sb = consts.tile([128, 1], BF)
            ones64 = consts.tile([1, 64], BF)
            nc.sync.dma_start(wq_sb[:], wq.rearrange("(cc p) m -> p cc m", p=128))
            nc.sync.dma_start(wk_sb[:], wk.rearrange("(cc p) m -> p cc m", p=128))
            nc.sync.dma_start(wv_sb[:], wv.rearrange("(cc p) m -> p cc m", p=128))
            nc.sync.dma_start(wp_sb[:], wps.rearrange("p (oc m) -> p oc m", m=128))
            nc.sync.dma_start(mb_sb[:], mb[:])
            nc.sync.dma_start(id_sb[:], ident[:])
            nc.vector.memset(ones_sb[:], 1.0)
            nc.vector.memset(ones64[:], 1.0)


            xb_tiles = {}
            kb_tiles = {}
            qkv_state = {}

            def emit_xb_load(b, first=False):
                xb = xpool.tile([128, CC, N], BF, name=f"xb{b}", tag="xb")
                xs = xT[:, b * N:(b + 1) * N].rearrange("(cc p) n -> p cc n", p=128)
                nk = nkcs[b] * KCH
                kb = kpool.tile([128, CC, nk], BF, name=f"kb{b}", tag="kb")
                ks = xTk[:, nko[b] * KCH:nko[b] * KCH + nk].rearrange(
                    "(cc p) n -> p cc n", p=128)
                if first:
                    # column-block-first order: q_unit(0)/k_unit(0) can start
                    # after ~1MB instead of after the full 6.25MB batch load
                    for cc in range(CC):
                        nc.sync.dma_start(kb[:, cc, 0:QB], ks[:, cc, 0:QB])
                    for cc in range(CC):
                        nc.sync.dma_start(xb[:, cc, 0:QB], xs[:, cc, 0:QB])
                    for cc in range(CC):
                        nc.sync.dma_start(kb[:, cc, QB:nk], ks[:, cc, QB:nk])
                    for rb in range(1, NQB):
                        for cc in range(CC):
                            nc.sync.dma_start(
                                xb[:, cc, rb * QB:(rb + 1) * QB],
                                xs[:, cc, rb * QB:(rb + 1) * QB],
                            )
                else:
                    # coarse loads: one dma_start each keeps the Sync engine
                    # (565ns per issue) nearly idle
                    nc.sync.dma_start(xb[:], xs)
                    nc.sync.dma_start(kb[:], ks)
                xb_tiles[b] = xb
                kb_tiles[b] = kb

            def qkv_units(b):
                """Independent emission units for batch b's QKV (filler work)."""
                xb = xb_tiles[b]
                kb = kb_tiles[b]
                nk = nkcs[b] * KCH
                qT = qkpool.tile([128, N], BF, name=f"qT{b}", tag="qT")
                kT = qkpool.tile([128, nk_max], BF, name=f"kT{b}", tag="kT")
                vt = vpool.tile([128, max(nkcs), CPC], BF, name=f"vt{b}", tag="vt")
                qkv_state[b] = (qT, kT, vt)
                units = []

                def k_unit(pos, w):
                    def emit():
                        ps = aux_ps.tile([128, QB], DT, name=f"psk{b}_{pos}", tag="aux")
                        for cc in range(CC):
                            nc.tensor.matmul(
                                ps[:, 0:w],
                                wk_sb[:, cc, :],
                                kb[:, cc, pos:pos + w],
                                start=cc == 0,
                                stop=cc == CC - 1,
                            )
                        nc.vector.tensor_copy(kT[:, pos:pos + w], ps[:, 0:w])

                    return emit

                def q_unit(rb):
                    def emit():
                        ps = aux_ps.tile([128, QB], DT, name=f"psq{b}_{rb}", tag="aux")
                        for cc in range(CC):
                            nc.tensor.matmul(
                                ps[:],
                                wq_sb[:, cc, :],
                                xb[:, cc, rb * QB:(rb + 1) * QB],
                                start=cc == 0,
                                stop=cc == CC - 1,
                            )
                        nc.vector.tensor_copy(qT[:, rb * QB:(rb + 1) * QB], ps[:])

                    return emit

                def v_unit(rc):
                    def emit():
                        ps = aux_ps.tile([128, QB], DT, name=f"psv{b}_{rc}", tag="aux")
                        for cc in range(CC):
                            nc.tensor.matmul(
                                ps[:, 0:CPC],
                                kb[:, cc, rc * KCH:(rc + 1) * KCH],
                                wv_sb[:, cc, :],
                                start=cc == 0,
                                stop=cc == CC - 1,
                            )
                        nc.vector.tensor_copy(vt[:, rc, :], ps[:, 0:CPC])

                    return emit

                # k first (scores need all of kT), then q blocks in order,
                # then v chunks (needed from the first O matmul onwards)
                for pos, w in kblocks_b[b]:
                    units.append(k_unit(pos, w))
                units.append(q_unit(0))
                for rc in range(nkcs[b]):
                    units.append(v_unit(rc))
                for rb in range(1, NQB):
                    units.append(q_unit(rb))
                return units

            def attention_steps(b, carried, extra=None):
                """Returns the woven list of emission closures for batch b.

                carried: trailing closures from the previous batch's last
                q-block (norm_b + proj units), woven into qb 0. extra:
                units appended to qb 0's trailers (run during qb 1).
                """
                qT, kT, vt = qkv_state[b]
                nkc = nkcs[b]
                o_tiles = {}
                d_tiles = {}
                e_tiles = {}
                nstate = {}
                of_tiles = {}

                def kc_scores(qb, kc):
                    def emit():
                        if kc == 0:
                            o_tiles[qb] = o_ps.tile(
                                [128, QB], DT, name=f"o{b}_{qb}", tag="o"
                            )
                            d_tiles[qb] = d_ps.tile(
                                [128, 256], DT, name=f"d{b}_{qb}", tag="d"
                            )
                        s2 = s_ps.tile(
                            [128, 2 * QB], DT, name=f"s{b}_{qb}_{kc}", tag="s"
                        )
                        nc.tensor.matmul(
                            s2[:, 0:QB],
                            kT[0:DH, kc * KCH:(kc + 1) * KCH],
                            qT[0:DH, qb * QB:(qb + 1) * QB],
                            start=True,
                            stop=True,
                            tile_position=(0, 0),
                        )
                        nc.tensor.matmul(
                            s2[:, QB:2 * QB],
                            kT[DH:2 * DH, kc * KCH:(kc + 1) * KCH],
                            qT[DH:2 * DH, qb * QB:(qb + 1) * QB],
                            start=True,
                            stop=True,
                            tile_position=(64, 0),
                        )
                        e2 = epool.tile(
                            [128, 2 * QB], BF, name=f"e{b}_{qb}_{kc}", tag="e"
                        )
                        mcol = nko[b] + kc
                        nc.scalar.activation(
                            e2[:],
                            s2[:],
                            mybir.ActivationFunctionType.Exp,
                            bias=mb_sb[:, mcol:mcol + 1],
                            scale=SCALE,
                        )
                        e_tiles[(qb, kc)] = e2

                    return emit

                def kc_ov(qb, kc):
                    def emit():
                        e2 = e_tiles.pop((qb, kc))
                        o_acc = o_tiles[qb]
                        d = d_tiles[qb]
                        st = kc == 0
                        sp = kc == nkc - 1
                        nc.tensor.matmul(
                            o_acc[0:DH, :],
                            vt[:, kc, 0:DH],
                            e2[:, 0:QB],
                            start=st,
                            stop=sp,
                            tile_position=(0, 0),
                        )
                        nc.tensor.matmul(
                            o_acc[DH:2 * DH, :],
                            vt[:, kc, DH:2 * DH],
                            e2[:, QB:2 * QB],
                            start=st,
                            stop=sp,
                            tile_position=(0, 64),
                        )
                        # PSUM start=True zeroes the whole 2KB bank row, so
                        # only the FIRST D group may set it; the rest inherit
                        # the bank-wide pending-zero (skip the group checker)
                        for h in range(2):
                            for qc in range(4):
                                col = h * 4 + qc
                                nc.tensor.matmul(
                                    d[:, col:col + 1],
                                    e2[:, h * QB + qc * 128:h * QB + (qc + 1) * 128],
                                    ones_sb[:],
                                    start=st and col == 0,
                                    stop=sp,
                                    skip_group_check=True,
                                )

                    return emit

                def norm_a(qb):
                    def emit():
                        # free the O PSUM bank immediately; reciprocal of D
                        o_acc = o_tiles.pop(qb)
                        d = d_tiles[qb]
                        rds = npool.tile([128, 8], DT, name=f"rds{b}_{qb}", tag="rds")
                        nc.vector.reciprocal_approx_fast(rds[:], d[:, 0:8])
                        osb = opool.tile([128, QB], DT, name=f"osb{b}_{qb}", tag="osb")
                        nc.vector.tensor_copy(osb[:], o_acc[:])
                        nstate[qb] = (osb, rds)

                    return emit

                def norm_t(qb):
                    def emit():
                        # 8 column transposes [128,1]->[1,128] in f32, all
                        # landing on partition 0 (matmul moving operands must
                        # start at an aligned partition);
                        # 4 regions per bank, assembled to one bf16 row
                        osb, rds = nstate[qb]
                        d_tiles.pop(qb)
                        rdsb = npool.tile(
                            [1, 8 * 128], BF, name=f"rdt{b}_{qb}", tag="rdt"
                        )
                        for half in range(2):
                            tt = aux_ps.tile(
                                [128, QB], DT, name=f"tt{b}_{qb}_{half}", tag="aux"
                            )
                            for i in range(4):
                                r = half * 4 + i
                                nc.tensor.transpose(
                                    tt[0:1, i * 128:(i + 1) * 128],
                                    rds[:, r:r + 1],
                                    id_sb[:],
                                )
                            nc.vector.tensor_copy(
                                rdsb[0:1, half * QB:(half + 1) * QB], tt[0:1, :]
                            )
                        nstate[qb] = (osb, rdsb)

                    return emit

                def norm_b(qb):
                    def emit():
                        osb, rdsb = nstate.pop(qb)
                        brd = aux_ps.tile([128, QB], DT, name=f"brd{b}_{qb}", tag="aux")
                        for qc in range(4):
                            nc.tensor.matmul(
                                brd[0:64, qc * 128:(qc + 1) * 128],
                                ones64[:],
                                rdsb[0:1, qc * 128:(qc + 1) * 128],
                                start=True,
                                stop=True,
                                tile_position=(0, 0),
                            )
                            nc.tensor.matmul(
                                brd[64:128, qc * 128:(qc + 1) * 128],
                                ones64[:],
                                rdsb[0:1, (4 + qc) * 128:(5 + qc) * 128],
                                start=True,
                                stop=True,
                                tile_position=(0, 64),
                            )
                        of = opool.tile([128, QB], BF, name=f"of{b}_{qb}", tag="of")
                        nc.vector.tensor_mul(of[:], osb[:], brd[:])
                        of_tiles[qb] = of
                        # per-qb output staging: 8 oc blocks, one DMA at the end
                        qkv_state[(b, qb, "ob")] = outp.tile(
                            [128, CC, QB], BF, name=f"ob{b}_{qb}", tag="ob"
                        )

                    return emit

                def proj_unit(qb, oc, last):
                    def emit():
                        of = of_tiles[qb] if not last else of_tiles.pop(qb)
                        ob = qkv_state[(b, qb, "ob")]
                        pps = aux_ps.tile(
                            [128, QB], DT, name=f"pp{b}_{qb}_{oc}", tag="aux"
                        )
                        nc.tensor.matmul(
                            pps[:], wp_sb[:, oc, :], of[:], start=True, stop=True
                        )
                        # split PSUM->SBUF copies between Vector and Scalar
                        if oc < 6:
                            nc.vector.tensor_copy(ob[:, oc, :], pps[:])
                        else:
                            nc.scalar.activation(
                                ob[:, oc, :],
                                pps[:],
                                mybir.ActivationFunctionType.Copy,
                            )
                        if last:
                            ob2 = qkv_state.pop((b, qb, "ob"))
                            nc.sync.dma_start(
                                out_ext[:, b * N + qb * QB:b * N + (qb + 1) * QB]
                                .rearrange("(oc p) q -> p oc q", p=128),
                                ob2[:],
                            )

                    return emit

                def qb_trailers(qb):
                    t = [norm_t(qb), norm_b(qb)]
                    for oc in range(CC):
                        t.append(proj_unit(qb, oc, oc == CC - 1))
                    if qb == 0 and extra:
                        t.extend(extra)
                    return t

                # weave: per qb emit scores(kc) with O/D lagged one step;
                # trailing work (norm_b + 8 proj units) of the PREVIOUS
                # q-block lands in slots kc=1..nkc-1 of this q-block.
                woven = []
                trailing = list(carried) if carried else []
                for qb in range(NQB):
                    for kc in range(nkc):
                        woven.append(kc_scores(qb, kc))
                        if kc >= 1:
                            woven.append(kc_ov(qb, kc - 1))
                            if trailing:
                                woven.append(trailing.pop(0))
                    # drain leftover trailing units BEFORE the O/D flush so
                    # the flush never head-of-line blocks on the last exp
                    while trailing:
                        woven.append(trailing.pop(0))
                    woven.append(kc_ov(qb, nkc - 1))
                    woven.append(norm_a(qb))
                    trailing = qb_trailers(qb)
                return woven, trailing

            def run_interleaved(steps, fillers):
                nf = len(fillers)
                ns = len(steps)
                fi = 0
                for i, s in enumerate(steps):
                    s()
                    while fi < nf and (i + 1) * nf >= (fi + 1) * ns:
                        fillers[fi]()
                        fi += 1
                while fi < nf:
                    fillers[fi]()
                    fi += 1

            # ---- schedule
            emit_xb_load(0, first=True)
            for u in qkv_units(0):
                u()
            carried = None
            for b in range(B):
                fillers = []
                if b < B - 1:
                    emit_xb_load(b + 1)
                    fillers.extend(qkv_units(b + 1))
                steps, carried = attention_steps(b, carried)
                run_interleaved(steps, fillers)
            # tail: last q-block's norm_b + proj
            for u in carried:
                u()

    nc.compile()
    return nc


def _prep_inputs(x, Wqkv, Wproj, bproj, mask, nkcs):
    x = np.asarray(x, dtype=np.float32)
    Wqkv = np.asarray(Wqkv, dtype=np.float32)
    Wproj = np.asarray(Wproj, dtype=np.float32)
    mask = np.asarray(mask)
    nkt = sum(nkcs)
    nko = [sum(nkcs[:b]) for b in range(B)]

    x2 = x.reshape(ROWS, C)
    xT = np.ascontiguousarray(x2.T).astype(NPBF)
    # compacted K/V tokens: unmasked columns per batch, zero-padded per batch
    xTk = np.zeros((C, nkt * KCH), dtype=NPBF)
    # mb columns: one per (batch, chunk); 128 rows = positions in the chunk
    mb_arr = np.full((128, nkt), np.float32(MASK_BIAS), dtype=np.float32)
    for b in range(B):
        idx = np.nonzero(mask[b] == 0)[0]
        cnt = len(idx)
        xTk[:, nko[b] * KCH: nko[b] * KCH + cnt] = xT[:, b * N + idx]
        mbias = np.full(nkcs[b] * KCH, np.float32(MASK_BIAS), dtype=np.float32)
        mbias[:cnt] = 0.0
        mb_arr[:, nko[b]:nko[b] + nkcs[b]] = mbias.reshape(nkcs[b], 128).T
    ident = np.eye(128, dtype=np.float32)

    in_maps = []
    for c in range(NCORES):
        cols = slice(c * CPC, (c + 1) * CPC)
        in_maps.append(
            dict(
                xT=xT,
                xTk=xTk,
                wq=np.ascontiguousarray(Wqkv[:, cols]).astype(NPBF),
                wk=np.ascontiguousarray(Wqkv[:, C:][:, cols]).astype(NPBF),
                wv=np.ascontiguousarray(Wqkv[:, 2 * C:][:, cols]).astype(NPBF),
                wps=np.ascontiguousarray(Wproj[cols, :]).astype(NPBF),
                mb=mb_arr,
                ident=ident,
            )
        )
    return in_maps


def kernel(x, Wqkv, Wproj, bproj, mask):
    global LAST_RESULTS
    mask = np.asarray(mask)
    bproj = np.asarray(bproj, dtype=np.float32)
    cnts = (mask == 0).sum(axis=1)
    nkcs = tuple(max(1, -(-int(c) // KCH)) for c in cnts)
    if nkcs not in _CACHE:
        _CACHE[nkcs] = _build(nkcs)
    nc = _CACHE[nkcs]
    in_maps = _prep_inputs(x, Wqkv, Wproj, bproj, mask, nkcs)
    res = run_bass_kernel_spmd(nc, in_maps, list(range(NCORES)))
    LAST_RESULTS = res
    # host-side unshard for row-parallel Wproj: sum the 8 partials + bias
    acc = np.zeros((C, ROWS), dtype=np.float32)
    for c in range(NCORES):
        acc += res.results[c]["out"].astype(np.float32)
    out = acc.T + bproj[None, :]
    return np.ascontiguousarray(out).reshape(B, N, C)
